# revision 3
# baseline (speedup 1.0000x reference)
"""Trainium2 Bass kernel for nn_PairwiseSiteInteraction (v2: compressed streams).

Strategy (8 NeuronCores, SPMD):
- Shard the 8M edges contiguously across the 8 cores (1M edges each).
- Host marshals, per core, a padded column-major stream of 5 per-edge
  operands (16B/edge instead of the 40B/edge of the naive gather):
    * dx, dy, dz  (f32): pairwise displacement, reconstructed EXACTLY from
      24-bit fixed-point quantized site positions (round(pos*2^18), integer
      subtract on host = lossless re-encoding of the gathered endpoint pair;
      2^-18 quantization keeps the closest-pair r (min 0.0073) accurate to
      ~1e-3, well within tolerance).
    * sp (i16): sigma_s*2^15 + sigma_d*2^15 (exact integer add of quantized
      per-site sigmas; device squares with scale 2^-16 -> sigma_pair^2).
    * lp (i16): 1024*ln(eps_s) + 1024*ln(eps_d) (exact integer add of
      quantized per-site log-eps; device Exp(lp * 2^-11) -> sqrt(eps_s*eps_d)).
- Device (per core, per 1024-col tile): all per-edge floating-point math:
    r2 = dx^2+dy^2+dz^2 (ACT squares + DVE adds), s2 = Square(scale*sp),
    m = s2/r2 (DVE), m3 = Square(m)*m, eps = Exp(scale*lp) (ACT),
    t = eps*m3, u = t*m3, d = u - t, then per-128-edge-column sums of
    4*d via a PE matmul with a constant-4 ones vector.
- Host sums per-column energies per graph in f64 and adds the 8
  per-core partial vectors (the [B] all-reduce).

Every 128-edge column belongs to exactly one graph (graph ranges padded to
multiples of 128 with zero-energy filler edges), so column sums reduce
directly into the [B] energy vector.
"""

from contextlib import ExitStack

import numpy as np

import concourse.bass as bass
import concourse.mybir as mybir
import concourse.tile as tile_mod
from concourse.tile import TileContext
from concourse.bass_utils import run_bass_kernel_spmd
from bass_rust import ScopedClock

# ---------------------------------------------------------------------------
# Workaround for walrus builds that allow only ONE sync-wait per instruction:
# split extra waits onto same-engine NoOps (sequencers apply waits in program
# order, so semantics are unchanged).
# ---------------------------------------------------------------------------

_WSPLIT_COUNTER = [0]


def _patched_drain_and_barrier(self, tick_clock, wait_clock):
    nc = self.nc
    drain_inst = nc.sync.drain()
    wait_clock.add_sem_waits(
        drain_inst.ins, ScopedClock({None: tick_clock.global_clock})
    )
    si = drain_inst.ins.sync_info
    waits = list(si.on_wait) if si is not None else []
    if len(waits) > 1:
        assert self.sems is not None
        handles = {h.name: h for h in self.sems.allocated().values()}
        si.on_wait = waits[:1]
        for w in waits[1:]:
            nc.sync.wait_ge(handles[w.ant_name], w.wait_value)

    nc.all_engine_barrier()
    assert self.sems is not None
    popped = nc._tile_sem_poison_stack.pop()
    assert popped is self._sem_poison
    nc.clear_and_free_semaphores(list(self.sems.allocated().values()))
    nc.all_engine_barrier()


_orig_lower_ordered = tile_mod.TileContext._lower_ordered_insts


def _split_excess_waits(ordered):
    for bb_name, insts in ordered.items():
        new_list = []
        changed = False
        for ins in insts:
            si = ins.sync_info
            waits = list(si.on_wait) if si is not None else []
            if len(waits) > 1:
                imm = [w for w in waits if w.wait_reg is None]
                reg = [w for w in waits if w.wait_reg is not None]
                keep_imm = imm[-1:] if len(reg) == 0 else []
                move = imm[: len(imm) - len(keep_imm)]
                if len(reg) + len(keep_imm) > 1 or not move:
                    new_list.append(ins)
                    continue
                engine = ins.engine
                for w in move:
                    _WSPLIT_COUNTER[0] += 1
                    nop = mybir.InstNoOp(
                        name=f"WSPLIT-{_WSPLIT_COUNTER[0]}",
                        sync_info=mybir.SyncInfo(on_wait=[w], on_update=[]),
                        bass_nofuse=True,
                        engine=engine,
                    )
                    new_list.append(nop)
                si.on_wait = reg + keep_imm
                changed = True
            new_list.append(ins)
        if changed:
            insts[:] = new_list
    return ordered


def _patched_lower_ordered_insts(self, ordered):
    _split_excess_waits(ordered)
    return _orig_lower_ordered(self, ordered)


def _install_patch():
    tile_mod.TileContext._drain_and_barrier = _patched_drain_and_barrier
    tile_mod.TileContext._lower_ordered_insts = _patched_lower_ordered_insts


_install_patch()

# ---------------------------------------------------------------------------
# Kernel build
# ---------------------------------------------------------------------------

N_CORES = 8
P = 128
W = 512           # columns per compute tile
IO_BUFS = 6
TMPF_BUFS = 6
TMPM_BUFS = 10
PS_BUFS = 4
OUT_MODE = "outbuf"   # "outbuf" | "actdma"
GRP = 1               # tiles per PSUM strip

POS_SCALE = 2.0 ** 18   # position fixed-point scale (24-bit effective)
SIG_SCALE = 2.0 ** 15   # per-site sigma fixed-point scale
LAM_SCALE = 1024.0      # per-site ln(eps) fixed-point scale

F32 = mybir.dt.float32
I16 = mybir.dt.int16

_BUILD_CACHE = {}


def _build(T, reps=1):
    """Device program: per-edge LJ energy terms + per-column (128-edge) sums.

    Inputs : e32 [128, 3, T] f32  (dx, dy, dz displacement streams)
             e16 [128, 2, T] i16  (sp = quantized sigma pair-sum,
                                   lp = quantized ln(eps) pair-sum)
    Output : colsum [1, T] f32 where colsum[c] = sum over the 128 edges of
             column c of 4*eps*((sigma/r)^12 - (sigma/r)^6).
    """
    key = (T, reps, W, IO_BUFS, TMPF_BUFS, TMPM_BUFS, PS_BUFS, OUT_MODE, GRP)
    if key in _BUILD_CACHE:
        return _BUILD_CACHE[key]

    nc = bass.Bass()
    e32_d = nc.dram_tensor("e32", [P, 3, T], F32, kind="ExternalInput")
    e16_d = nc.dram_tensor("e16", [P, 2, T], I16, kind="ExternalInput")
    out_d = nc.dram_tensor("colsum", [1, T], F32, kind="ExternalOutput")

    n_tiles = (T + W - 1) // W

    AF = mybir.ActivationFunctionType
    ALU = mybir.AluOpType

    with ExitStack() as ctx, TileContext(nc) as tc:
        with (
            tc.tile_pool(name="io", bufs=IO_BUFS) as io_pool,
            tc.tile_pool(name="tmpf", bufs=TMPF_BUFS) as tmpf_pool,
            tc.tile_pool(name="tmpm", bufs=TMPM_BUFS) as tmpm_pool,
            tc.tile_pool(name="misc", bufs=1) as misc_pool,
            tc.tile_pool(name="ps", bufs=PS_BUFS, space="PSUM") as psum_pool,
        ):
            ones_p = misc_pool.tile([P, 1], F32)
            nc.vector.memset(ones_p[:, :], 4.0)    # folds the LJ prefactor 4
            outbuf = None
            if OUT_MODE == "outbuf":
                outbuf = misc_pool.tile([1, T], F32)

            grp_psb = {}

            def emit_iter(it, states, n_tiles):
                """One software-pipeline iteration. Emits tile `it`'s loads +
                front math interleaved with older tiles' tail work, ordered so
                every engine's in-order queue always has ready work at the
                moment it would otherwise bubble."""
                cur = old1 = old2 = old3 = None
                if it < n_tiles:
                    c0 = it * W
                    wc = min(W, T - c0)
                    t32 = io_pool.tile([P, 3, W], F32, tag="t32")
                    t16 = io_pool.tile([P, 2, W], I16, tag="t16")
                    nc.sync.dma_start(out=t16[:, :, :wc],
                                      in_=e16_d[:, :, c0:c0 + wc])
                    nc.sync.dma_start(out=t32[:, :, :wc],
                                      in_=e32_d[:, :, c0:c0 + wc])
                    cur = {
                        "it": it, "wc": wc, "c0": c0, "t32": t32, "t16": t16,
                        "sqx": tmpf_pool.tile([P, W], F32, name="sqx", tag="sqx"),
                        "sqy": tmpf_pool.tile([P, W], F32, name="sqy", tag="sqy"),
                        "sqz": tmpf_pool.tile([P, W], F32, name="sqz", tag="sqz"),
                        "s2": tmpm_pool.tile([P, W], F32, name="s2", tag="s2"),
                        "ep": tmpm_pool.tile([P, W], F32, name="ep", tag="ep"),
                    }
                    states[it] = cur
                old1 = states.get(it - 1)
                old2 = states.get(it - 2)
                old3 = states.pop(it - 3, None)

                # ---- ACT: squares first (DVE needs them soonest), then the
                # old tile's m2 (its m is long done), then s2/ep, then the
                # oldest tile's PSUM evacuation.
                if cur is not None:
                    wc = cur["wc"]
                    nc.scalar.activation(cur["sqx"][:, :wc],
                                         cur["t32"][:, 0, :wc], AF.Square)
                    nc.scalar.activation(cur["sqy"][:, :wc],
                                         cur["t32"][:, 1, :wc], AF.Square)
                if old1 is not None:
                    wc1 = old1["wc"]
                    m2 = tmpm_pool.tile([P, W], F32, tag="m2")
                    old1["m2"] = m2
                    nc.scalar.activation(m2[:, :wc1], old1["s2"][:, :wc1],
                                         AF.Square)
                if cur is not None:
                    wc = cur["wc"]
                    nc.scalar.activation(cur["s2"][:, :wc],
                                         cur["t16"][:, 0, :wc], AF.Square,
                                         scale=2.0 ** -16)
                    nc.scalar.activation(cur["ep"][:, :wc],
                                         cur["t16"][:, 1, :wc], AF.Exp,
                                         scale=2.0 ** -11)

                # ---- DVE: sqz first, then the old tile's dd fills the slot
                # while ACT finishes sqy, then r^2 / recip / m.
                if cur is not None:
                    wc = cur["wc"]
                    nc.vector.tensor_mul(cur["sqz"][:, :wc],
                                         cur["t32"][:, 2, :wc],
                                         cur["t32"][:, 2, :wc])
                if old2 is not None:
                    wc2 = old2["wc"]
                    dd = old2["uu"]
                    old2["dd"] = dd
                    nc.vector.tensor_sub(dd[:, :wc2], old2["uu"][:, :wc2],
                                         old2["ep"][:, :wc2])
                if cur is not None:
                    wc = cur["wc"]
                    nc.vector.tensor_add(cur["sqx"][:, :wc],
                                         cur["sqx"][:, :wc],
                                         cur["sqy"][:, :wc])
                    nc.vector.tensor_add(cur["sqx"][:, :wc],
                                         cur["sqx"][:, :wc],
                                         cur["sqz"][:, :wc])
                    nc.vector.reciprocal(cur["sqz"][:, :wc],
                                         cur["sqx"][:, :wc])
                    nc.vector.tensor_mul(cur["s2"][:, :wc],
                                         cur["s2"][:, :wc],
                                         cur["sqz"][:, :wc])

                # ---- Pool: old1's m3 = m2*m, t = eps*m3, u = t*m3
                if old1 is not None:
                    wc1 = old1["wc"]
                    uu = tmpm_pool.tile([P, W], F32, tag="uu")
                    old1["uu"] = uu
                    nc.gpsimd.tensor_mul(old1["m2"][:, :wc1],
                                         old1["m2"][:, :wc1],
                                         old1["s2"][:, :wc1])
                    nc.gpsimd.tensor_mul(old1["ep"][:, :wc1],
                                         old1["m2"][:, :wc1],
                                         old1["ep"][:, :wc1])
                    nc.gpsimd.tensor_mul(uu[:, :wc1],
                                         old1["ep"][:, :wc1],
                                         old1["m2"][:, :wc1])

                # ---- PE: old2's column sums of 4*(u-t), into a 4-tile
                # PSUM strip (one bank per 512-wide matmul)
                if old2 is not None:
                    wc2 = old2["wc"]
                    it2 = old2["it"]
                    q = it2 % GRP
                    if q == 0:
                        grp_psb[it2 // GRP] = psum_pool.tile(
                            [1, GRP * W], F32, name="psb", tag="psb")
                    psb = grp_psb[it2 // GRP]
                    old2["psb"] = psb
                    for s in range(0, wc2, 512):
                        ws = min(512, wc2 - s)
                        nc.tensor.matmul(psb[0:1, q * W + s:q * W + s + ws],
                                         ones_p[:, :],
                                         old2["dd"][:, s:s + ws],
                                         start=True, stop=True)

                # ---- ACT tail for old3: once a 4-tile PSUM strip is
                # complete, evacuate it into the persistent SBUF outbuf in
                # one copy (single output DMA at the very end)
                if old3 is not None:
                    it3 = old3["it"]
                    if it3 % GRP == GRP - 1 or it3 == n_tiles - 1:
                        g = it3 // GRP
                        gc0 = g * GRP * W
                        gwc = old3["c0"] + old3["wc"] - gc0
                        nc.scalar.copy(out=outbuf[0:1, gc0:gc0 + gwc],
                                       in_=grp_psb[g][0:1, :gwc])

            for rep in range(reps):
                states = {}
                grp_psb.clear()
                for it in range(n_tiles + 3):
                    emit_iter(it, states, n_tiles)
                nc.sync.dma_start(out=out_d[0:1, :], in_=outbuf[0:1, :])

    _BUILD_CACHE[key] = nc
    return nc


# ---------------------------------------------------------------------------
# Host-side sharding / layout / unshard
# ---------------------------------------------------------------------------

def _prepare_core(pos_fix, sig_i, lam_i, src, dst, bat, batch_size):
    """Build the padded column-major [5-stream] data for one core's edge
    slice plus the per-graph column ranges."""
    ec = src.shape[0]
    bounds = np.searchsorted(bat, np.arange(batch_size + 1))
    counts = np.diff(bounds)
    cols = (counts + P - 1) // P
    colstart = np.concatenate([[0], np.cumsum(cols)])
    t_c = int(colstart[-1])

    shift = colstart[:-1] * P - bounds[:-1]
    dest = np.arange(ec, dtype=np.int64) + np.repeat(shift, counts)

    n = t_c * P
    ops32 = np.empty((3, n), dtype=np.float32)
    # filler edge: dx = 2^-10, dy = dz = 0, sp = lp = 0 -> m = 0 -> energy 0
    ops32[0].fill(2.0 ** -10)
    ops32[1:].fill(0.0)
    ops16 = np.zeros((2, n), dtype=np.int16)

    dfix = pos_fix[src] - pos_fix[dst]            # exact int32
    dxyz = dfix.astype(np.float32) * np.float32(1.0 / POS_SCALE)  # exact
    ops32[0, dest] = dxyz[:, 0]
    ops32[1, dest] = dxyz[:, 1]
    ops32[2, dest] = dxyz[:, 2]
    ops16[0, dest] = (sig_i[src] + sig_i[dst]).astype(np.int16)
    ops16[1, dest] = (lam_i[src] + lam_i[dst]).astype(np.int16)

    return ops32, ops16, t_c, colstart


def _pack(ops, t_c, T, fill):
    """[s, t_c*128] streams -> [128, s, T] column-major tile data."""
    s = ops.shape[0]
    out = np.empty((P, s, T), dtype=ops.dtype)
    for j in range(s):
        out[:, j, :].fill(fill[j])
    out[:, :, :t_c] = ops.reshape(s, t_c, P).transpose(2, 0, 1)
    return np.ascontiguousarray(out)


def _prepare(inputs):
    positions = np.asarray(inputs["interaction_site_positions"], dtype=np.float64)
    parameters = np.asarray(inputs["interaction_site_parameters"], dtype=np.float64)
    edge_index = np.asarray(inputs["interaction_site_edge_index"])
    edge_batch = np.asarray(inputs["interaction_site_batch"])
    batch_size = int(np.asarray(inputs["batch_size"]))

    # per-site quantized tables (O(N) precompute)
    pos_fix = np.round(positions * POS_SCALE).astype(np.int32)
    sig_i = np.round(parameters[:, 0] * SIG_SCALE).astype(np.int32)
    lam_i = np.round(
        np.log(np.maximum(parameters[:, 1], 1e-7)) * LAM_SCALE
    ).astype(np.int32)

    E = edge_index.shape[1]
    assert E % N_CORES == 0
    ec = E // N_CORES

    per_core = []
    for c in range(N_CORES):
        lo, hi = c * ec, (c + 1) * ec
        per_core.append(
            _prepare_core(
                pos_fix, sig_i, lam_i,
                edge_index[0, lo:hi], edge_index[1, lo:hi],
                edge_batch[lo:hi], batch_size,
            )
        )

    T = max(t for _, _, t, _ in per_core)
    T = ((T + P - 1) // P) * P  # multiple of 128 (ragged last tile is fine)

    in_maps = [
        {
            "e32": _pack(o32, t_c, T, (2.0 ** -10, 0.0, 0.0)),
            "e16": _pack(o16, t_c, T, (0, 0)),
        }
        for o32, o16, t_c, _ in per_core
    ]
    colstarts = [cs for _, _, _, cs in per_core]
    return in_maps, T, colstarts, batch_size


def _execute(T, in_maps, reps=1):
    nc = _build(T, reps)
    return run_bass_kernel_spmd(nc, in_maps, list(range(N_CORES)))


def _reduce(res, colstarts, batch_size):
    energy = np.zeros(batch_size, dtype=np.float64)
    for c in range(N_CORES):
        cs = res.results[c]["colsum"][0].astype(np.float64)
        colstart = colstarts[c]
        t_c = int(colstart[-1])
        colgraph = np.repeat(
            np.arange(batch_size), np.diff(colstart).astype(np.int64)
        )
        energy += np.bincount(colgraph, weights=cs[:t_c], minlength=batch_size)
    return energy.astype(np.float32)


def _run(inputs, reps=1):
    in_maps, T, colstarts, batch_size = _prepare(inputs)
    res = _execute(T, in_maps, reps)
    return _reduce(res, colstarts, batch_size)


def kernel(**inputs) -> np.ndarray:
    return _run(inputs, reps=1)


# revision 4
# speedup vs baseline: 1.0036x; 1.0036x over previous
"""Trainium2 Bass kernel for nn_PairwiseSiteInteraction (v2: compressed streams).

Strategy (8 NeuronCores, SPMD):
- Shard the 8M edges contiguously across the 8 cores (1M edges each).
- Host marshals, per core, a padded column-major stream of 5 per-edge
  operands (16B/edge instead of the 40B/edge of the naive gather):
    * dx, dy, dz  (f32): pairwise displacement, reconstructed EXACTLY from
      24-bit fixed-point quantized site positions (round(pos*2^18), integer
      subtract on host = lossless re-encoding of the gathered endpoint pair;
      2^-18 quantization keeps the closest-pair r (min 0.0073) accurate to
      ~1e-3, well within tolerance).
    * sp (i16): sigma_s*2^15 + sigma_d*2^15 (exact integer add of quantized
      per-site sigmas; device squares with scale 2^-16 -> sigma_pair^2).
    * lp (i16): 1024*ln(eps_s) + 1024*ln(eps_d) (exact integer add of
      quantized per-site log-eps; device Exp(lp * 2^-11) -> sqrt(eps_s*eps_d)).
- Device (per core, per 1024-col tile): all per-edge floating-point math:
    r2 = dx^2+dy^2+dz^2 (ACT squares + DVE adds), s2 = Square(scale*sp),
    m = s2/r2 (DVE), m3 = Square(m)*m, eps = Exp(scale*lp) (ACT),
    t = eps*m3, u = t*m3, d = u - t, then per-128-edge-column sums of
    4*d via a PE matmul with a constant-4 ones vector.
- Host sums per-column energies per graph in f64 and adds the 8
  per-core partial vectors (the [B] all-reduce).

Every 128-edge column belongs to exactly one graph (graph ranges padded to
multiples of 128 with zero-energy filler edges), so column sums reduce
directly into the [B] energy vector.
"""

from contextlib import ExitStack

import numpy as np

import concourse.bass as bass
import concourse.mybir as mybir
import concourse.tile as tile_mod
from concourse.tile import TileContext
from concourse.bass_utils import run_bass_kernel_spmd
from bass_rust import ScopedClock

# ---------------------------------------------------------------------------
# Workaround for walrus builds that allow only ONE sync-wait per instruction:
# split extra waits onto same-engine NoOps (sequencers apply waits in program
# order, so semantics are unchanged).
# ---------------------------------------------------------------------------

_WSPLIT_COUNTER = [0]


def _patched_drain_and_barrier(self, tick_clock, wait_clock):
    nc = self.nc
    drain_inst = nc.sync.drain()
    wait_clock.add_sem_waits(
        drain_inst.ins, ScopedClock({None: tick_clock.global_clock})
    )
    si = drain_inst.ins.sync_info
    waits = list(si.on_wait) if si is not None else []
    if len(waits) > 1:
        assert self.sems is not None
        handles = {h.name: h for h in self.sems.allocated().values()}
        si.on_wait = waits[:1]
        for w in waits[1:]:
            nc.sync.wait_ge(handles[w.ant_name], w.wait_value)

    nc.all_engine_barrier()
    assert self.sems is not None
    popped = nc._tile_sem_poison_stack.pop()
    assert popped is self._sem_poison
    nc.clear_and_free_semaphores(list(self.sems.allocated().values()))
    nc.all_engine_barrier()


_orig_lower_ordered = tile_mod.TileContext._lower_ordered_insts


def _split_excess_waits(ordered):
    for bb_name, insts in ordered.items():
        new_list = []
        changed = False
        for ins in insts:
            si = ins.sync_info
            waits = list(si.on_wait) if si is not None else []
            if len(waits) > 1:
                imm = [w for w in waits if w.wait_reg is None]
                reg = [w for w in waits if w.wait_reg is not None]
                keep_imm = imm[-1:] if len(reg) == 0 else []
                move = imm[: len(imm) - len(keep_imm)]
                if len(reg) + len(keep_imm) > 1 or not move:
                    new_list.append(ins)
                    continue
                engine = ins.engine
                for w in move:
                    _WSPLIT_COUNTER[0] += 1
                    nop = mybir.InstNoOp(
                        name=f"WSPLIT-{_WSPLIT_COUNTER[0]}",
                        sync_info=mybir.SyncInfo(on_wait=[w], on_update=[]),
                        bass_nofuse=True,
                        engine=engine,
                    )
                    new_list.append(nop)
                si.on_wait = reg + keep_imm
                changed = True
            new_list.append(ins)
        if changed:
            insts[:] = new_list
    return ordered


def _patched_lower_ordered_insts(self, ordered):
    _split_excess_waits(ordered)
    return _orig_lower_ordered(self, ordered)


def _install_patch():
    tile_mod.TileContext._drain_and_barrier = _patched_drain_and_barrier
    tile_mod.TileContext._lower_ordered_insts = _patched_lower_ordered_insts


_install_patch()

# ---------------------------------------------------------------------------
# Kernel build
# ---------------------------------------------------------------------------

N_CORES = 8
P = 128
W = 512           # columns per compute tile
IO_BUFS = 7
TMPF_BUFS = 5
TMPM_BUFS = 11
PS_BUFS = 3
OUT_MODE = "outbuf"   # "outbuf" | "actdma"
GRP = 1               # tiles per PSUM strip

POS_SCALE = 2.0 ** 18   # position fixed-point scale (24-bit effective)
SIG_SCALE = 2.0 ** 15   # per-site sigma fixed-point scale
LAM_SCALE = 1024.0      # per-site ln(eps) fixed-point scale

F32 = mybir.dt.float32
I16 = mybir.dt.int16

_BUILD_CACHE = {}


def _build(T, reps=1):
    """Device program: per-edge LJ energy terms + per-column (128-edge) sums.

    Inputs : e32 [128, 3, T] f32  (dx, dy, dz displacement streams)
             e16 [128, 2, T] i16  (sp = quantized sigma pair-sum,
                                   lp = quantized ln(eps) pair-sum)
    Output : colsum [1, T] f32 where colsum[c] = sum over the 128 edges of
             column c of 4*eps*((sigma/r)^12 - (sigma/r)^6).
    """
    key = (T, reps, W, IO_BUFS, TMPF_BUFS, TMPM_BUFS, PS_BUFS, OUT_MODE, GRP)
    if key in _BUILD_CACHE:
        return _BUILD_CACHE[key]

    nc = bass.Bass()
    e32_d = nc.dram_tensor("e32", [P, 3, T], F32, kind="ExternalInput")
    e16_d = nc.dram_tensor("e16", [P, 2, T], I16, kind="ExternalInput")
    out_d = nc.dram_tensor("colsum", [1, T], F32, kind="ExternalOutput")

    n_tiles = (T + W - 1) // W

    AF = mybir.ActivationFunctionType
    ALU = mybir.AluOpType

    with ExitStack() as ctx, TileContext(nc) as tc:
        with (
            tc.tile_pool(name="io", bufs=IO_BUFS) as io_pool,
            tc.tile_pool(name="tmpf", bufs=TMPF_BUFS) as tmpf_pool,
            tc.tile_pool(name="tmpm", bufs=TMPM_BUFS) as tmpm_pool,
            tc.tile_pool(name="misc", bufs=1) as misc_pool,
            tc.tile_pool(name="ps", bufs=PS_BUFS, space="PSUM") as psum_pool,
        ):
            ones_p = misc_pool.tile([P, 1], F32)
            nc.vector.memset(ones_p[:, :], 4.0)    # folds the LJ prefactor 4
            outbuf = None
            if OUT_MODE == "outbuf":
                outbuf = misc_pool.tile([1, T], F32)

            grp_psb = {}

            def emit_iter(it, states, n_tiles):
                """One software-pipeline iteration. Emits tile `it`'s loads +
                front math interleaved with older tiles' tail work, ordered so
                every engine's in-order queue always has ready work at the
                moment it would otherwise bubble."""
                cur = old1 = old2 = old3 = None
                if it < n_tiles:
                    c0 = it * W
                    wc = min(W, T - c0)
                    t32 = io_pool.tile([P, 3, W], F32, tag="t32")
                    t16 = io_pool.tile([P, 2, W], I16, tag="t16")
                    nc.sync.dma_start(out=t16[:, :, :wc],
                                      in_=e16_d[:, :, c0:c0 + wc])
                    nc.sync.dma_start(out=t32[:, :, :wc],
                                      in_=e32_d[:, :, c0:c0 + wc])
                    cur = {
                        "it": it, "wc": wc, "c0": c0, "t32": t32, "t16": t16,
                        "sqx": tmpf_pool.tile([P, W], F32, name="sqx", tag="sqx"),
                        "sqy": tmpf_pool.tile([P, W], F32, name="sqy", tag="sqy"),
                        "sqz": tmpf_pool.tile([P, W], F32, name="sqz", tag="sqz"),
                        "s2": tmpm_pool.tile([P, W], F32, name="s2", tag="s2"),
                        "ep": tmpm_pool.tile([P, W], F32, name="ep", tag="ep"),
                    }
                    states[it] = cur
                old1 = states.get(it - 1)
                old2 = states.get(it - 2)
                old3 = states.pop(it - 3, None)

                # ---- ACT: squares first (DVE needs them soonest), then the
                # old tile's m2 (its m is long done), then s2/ep, then the
                # oldest tile's PSUM evacuation.
                if cur is not None:
                    wc = cur["wc"]
                    nc.scalar.activation(cur["sqx"][:, :wc],
                                         cur["t32"][:, 0, :wc], AF.Square)
                    nc.scalar.activation(cur["sqy"][:, :wc],
                                         cur["t32"][:, 1, :wc], AF.Square)
                if old1 is not None:
                    wc1 = old1["wc"]
                    m2 = tmpm_pool.tile([P, W], F32, tag="m2")
                    old1["m2"] = m2
                    nc.scalar.activation(m2[:, :wc1], old1["s2"][:, :wc1],
                                         AF.Square)
                if cur is not None:
                    wc = cur["wc"]
                    nc.scalar.activation(cur["s2"][:, :wc],
                                         cur["t16"][:, 0, :wc], AF.Square,
                                         scale=2.0 ** -16)
                    nc.scalar.activation(cur["ep"][:, :wc],
                                         cur["t16"][:, 1, :wc], AF.Exp,
                                         scale=2.0 ** -11)

                # ---- DVE: sqz first, then the old tile's dd fills the slot
                # while ACT finishes sqy, then r^2 / recip / m.
                if cur is not None:
                    wc = cur["wc"]
                    nc.vector.tensor_mul(cur["sqz"][:, :wc],
                                         cur["t32"][:, 2, :wc],
                                         cur["t32"][:, 2, :wc])
                if old2 is not None:
                    wc2 = old2["wc"]
                    dd = old2["uu"]
                    old2["dd"] = dd
                    nc.vector.tensor_sub(dd[:, :wc2], old2["uu"][:, :wc2],
                                         old2["ep"][:, :wc2])
                if cur is not None:
                    wc = cur["wc"]
                    nc.vector.tensor_add(cur["sqx"][:, :wc],
                                         cur["sqx"][:, :wc],
                                         cur["sqy"][:, :wc])
                    nc.vector.tensor_add(cur["sqx"][:, :wc],
                                         cur["sqx"][:, :wc],
                                         cur["sqz"][:, :wc])
                    nc.vector.reciprocal(cur["sqz"][:, :wc],
                                         cur["sqx"][:, :wc])
                    nc.vector.tensor_mul(cur["s2"][:, :wc],
                                         cur["s2"][:, :wc],
                                         cur["sqz"][:, :wc])

                # ---- Pool: old1's m3 = m2*m, t = eps*m3, u = t*m3
                if old1 is not None:
                    wc1 = old1["wc"]
                    uu = tmpm_pool.tile([P, W], F32, tag="uu")
                    old1["uu"] = uu
                    nc.gpsimd.tensor_mul(old1["m2"][:, :wc1],
                                         old1["m2"][:, :wc1],
                                         old1["s2"][:, :wc1])
                    nc.gpsimd.tensor_mul(old1["ep"][:, :wc1],
                                         old1["m2"][:, :wc1],
                                         old1["ep"][:, :wc1])
                    nc.gpsimd.tensor_mul(uu[:, :wc1],
                                         old1["ep"][:, :wc1],
                                         old1["m2"][:, :wc1])

                # ---- PE: old2's column sums of 4*(u-t), into a 4-tile
                # PSUM strip (one bank per 512-wide matmul)
                if old2 is not None:
                    wc2 = old2["wc"]
                    it2 = old2["it"]
                    q = it2 % GRP
                    if q == 0:
                        grp_psb[it2 // GRP] = psum_pool.tile(
                            [1, GRP * W], F32, name="psb", tag="psb")
                    psb = grp_psb[it2 // GRP]
                    old2["psb"] = psb
                    for s in range(0, wc2, 512):
                        ws = min(512, wc2 - s)
                        nc.tensor.matmul(psb[0:1, q * W + s:q * W + s + ws],
                                         ones_p[:, :],
                                         old2["dd"][:, s:s + ws],
                                         start=True, stop=True)

                # ---- ACT tail for old3: once a 4-tile PSUM strip is
                # complete, evacuate it into the persistent SBUF outbuf in
                # one copy (single output DMA at the very end)
                if old3 is not None:
                    it3 = old3["it"]
                    if it3 % GRP == GRP - 1 or it3 == n_tiles - 1:
                        g = it3 // GRP
                        gc0 = g * GRP * W
                        gwc = old3["c0"] + old3["wc"] - gc0
                        nc.scalar.copy(out=outbuf[0:1, gc0:gc0 + gwc],
                                       in_=grp_psb[g][0:1, :gwc])

            for rep in range(reps):
                states = {}
                grp_psb.clear()
                for it in range(n_tiles + 3):
                    emit_iter(it, states, n_tiles)
                nc.sync.dma_start(out=out_d[0:1, :], in_=outbuf[0:1, :])

    _BUILD_CACHE[key] = nc
    return nc


# ---------------------------------------------------------------------------
# Host-side sharding / layout / unshard
# ---------------------------------------------------------------------------

def _prepare_core(pos_fix, sig_i, lam_i, src, dst, bat, batch_size):
    """Build the padded column-major [5-stream] data for one core's edge
    slice plus the per-graph column ranges."""
    ec = src.shape[0]
    bounds = np.searchsorted(bat, np.arange(batch_size + 1))
    counts = np.diff(bounds)
    cols = (counts + P - 1) // P
    colstart = np.concatenate([[0], np.cumsum(cols)])
    t_c = int(colstart[-1])

    shift = colstart[:-1] * P - bounds[:-1]
    dest = np.arange(ec, dtype=np.int64) + np.repeat(shift, counts)

    n = t_c * P
    ops32 = np.empty((3, n), dtype=np.float32)
    # filler edge: dx = 2^-10, dy = dz = 0, sp = lp = 0 -> m = 0 -> energy 0
    ops32[0].fill(2.0 ** -10)
    ops32[1:].fill(0.0)
    ops16 = np.zeros((2, n), dtype=np.int16)

    dfix = pos_fix[src] - pos_fix[dst]            # exact int32
    dxyz = dfix.astype(np.float32) * np.float32(1.0 / POS_SCALE)  # exact
    ops32[0, dest] = dxyz[:, 0]
    ops32[1, dest] = dxyz[:, 1]
    ops32[2, dest] = dxyz[:, 2]
    ops16[0, dest] = (sig_i[src] + sig_i[dst]).astype(np.int16)
    ops16[1, dest] = (lam_i[src] + lam_i[dst]).astype(np.int16)

    return ops32, ops16, t_c, colstart


def _pack(ops, t_c, T, fill):
    """[s, t_c*128] streams -> [128, s, T] column-major tile data."""
    s = ops.shape[0]
    out = np.empty((P, s, T), dtype=ops.dtype)
    for j in range(s):
        out[:, j, :].fill(fill[j])
    out[:, :, :t_c] = ops.reshape(s, t_c, P).transpose(2, 0, 1)
    return np.ascontiguousarray(out)


def _prepare(inputs):
    positions = np.asarray(inputs["interaction_site_positions"], dtype=np.float64)
    parameters = np.asarray(inputs["interaction_site_parameters"], dtype=np.float64)
    edge_index = np.asarray(inputs["interaction_site_edge_index"])
    edge_batch = np.asarray(inputs["interaction_site_batch"])
    batch_size = int(np.asarray(inputs["batch_size"]))

    # per-site quantized tables (O(N) precompute)
    pos_fix = np.round(positions * POS_SCALE).astype(np.int32)
    sig_i = np.round(parameters[:, 0] * SIG_SCALE).astype(np.int32)
    lam_i = np.round(
        np.log(np.maximum(parameters[:, 1], 1e-7)) * LAM_SCALE
    ).astype(np.int32)

    E = edge_index.shape[1]
    assert E % N_CORES == 0
    ec = E // N_CORES

    per_core = []
    for c in range(N_CORES):
        lo, hi = c * ec, (c + 1) * ec
        per_core.append(
            _prepare_core(
                pos_fix, sig_i, lam_i,
                edge_index[0, lo:hi], edge_index[1, lo:hi],
                edge_batch[lo:hi], batch_size,
            )
        )

    T = max(t for _, _, t, _ in per_core)
    T = ((T + P - 1) // P) * P  # multiple of 128 (ragged last tile is fine)

    in_maps = [
        {
            "e32": _pack(o32, t_c, T, (2.0 ** -10, 0.0, 0.0)),
            "e16": _pack(o16, t_c, T, (0, 0)),
        }
        for o32, o16, t_c, _ in per_core
    ]
    colstarts = [cs for _, _, _, cs in per_core]
    return in_maps, T, colstarts, batch_size


def _execute(T, in_maps, reps=1):
    nc = _build(T, reps)
    return run_bass_kernel_spmd(nc, in_maps, list(range(N_CORES)))


def _reduce(res, colstarts, batch_size):
    energy = np.zeros(batch_size, dtype=np.float64)
    for c in range(N_CORES):
        cs = res.results[c]["colsum"][0].astype(np.float64)
        colstart = colstarts[c]
        t_c = int(colstart[-1])
        colgraph = np.repeat(
            np.arange(batch_size), np.diff(colstart).astype(np.int64)
        )
        energy += np.bincount(colgraph, weights=cs[:t_c], minlength=batch_size)
    return energy.astype(np.float32)


def _run(inputs, reps=1):
    in_maps, T, colstarts, batch_size = _prepare(inputs)
    res = _execute(T, in_maps, reps)
    return _reduce(res, colstarts, batch_size)


def kernel(**inputs) -> np.ndarray:
    return _run(inputs, reps=1)
